# revision 13
# baseline (speedup 1.0000x reference)
"""Trainium2 Bass kernel for DynamicChannelExchangeWithSE.

Contract: kernel(**inputs) takes the FULL unsharded inputs (numpy, keyed as in
setup_inputs) and returns the full (out_lst, out_gui, m) tuple.

Strategy (8 NeuronCores, H sharded 8-ways -> per-core [128, 64*512] slabs):
  Phase A  stream both slabs once; GpSimd accumulates per-channel partial
           sums. The last STASH tiles per slab stay resident in SBUF.
  AllGather the [128, 2] partial sums across the 8 cores (1 KB payload),
           combine on DVE.
  Mask     tiny on-device pipeline: FCNet MLP + SE MLP (PE matmuls, DVE
           relu, ACT sigmoid), top-64 selection by rank counting (PE
           outer-product broadcast + DVE compare/reduce), ranks via prefix
           scan, selection matrix P, then A2T = P conv2_w^T P^T etc.
  Phase B  process stashed tiles first (no DMA), then re-stream the rest;
           per 512-col chunk:
              out_lst = (lst*(1-s) + beta2) + A2T.T @ gui
              out_gui = (gui*(1-s) + beta1) + A1T.T @ lst
           (PE fp32 matmul into PSUM; DVE tensor_scalar folds the
           passthrough+bias; DVE tensor_tensor adds the PSUM.)
"""

import numpy as np

C = 128          # channels per tensor
HALF = 64        # selected channel count / conv dims
MD = 64          # mask input dim
SE_HID = 16
H = 512
W = 512
NCORES = 8
HSHARD = H // NCORES          # 64 rows per core
FREE = HSHARD * W             # 32768 columns per core


def build_nc(free=FREE, tile=2048, mm=512, ncores=NCORES, n_stash=6,
             use_f32r=True, use_remote=True):
    """Build the per-core Bass program. All cores run the identical program."""
    import concourse.bacc as bacc
    import concourse.bass as bass
    import concourse.mybir as mybir
    from concourse import tile as tile_mod
    from concourse.alu_op_type import AluOpType
    from bass_rust import AxisListType

    f32 = mybir.dt.float32
    f32r = mybir.dt.float32r if use_f32r else mybir.dt.float32
    AF = mybir.ActivationFunctionType
    nt = free // tile
    assert tile % mm == 0 and free % tile == 0
    assert 0 <= n_stash < nt
    mm_per_tile = tile // mm

    nc = bacc.Bacc("TRN2", target_bir_lowering=False, debug=False,
                   num_devices=ncores)

    lst = nc.dram_tensor("lst", [C, free], f32r, kind="ExternalInput")
    gui = nc.dram_tensor("gui", [C, free], f32r, kind="ExternalInput")
    maskv = nc.dram_tensor("maskv", [MD, 1], f32, kind="ExternalInput")
    w1 = nc.dram_tensor("w1", [MD, C], f32, kind="ExternalInput")
    b1 = nc.dram_tensor("b1", [C, 1], f32, kind="ExternalInput")
    w2 = nc.dram_tensor("w2", [C, C], f32, kind="ExternalInput")
    b2 = nc.dram_tensor("b2", [C, 1], f32, kind="ExternalInput")
    sew1l = nc.dram_tensor("sew1l", [C, SE_HID], f32, kind="ExternalInput")
    sew1g = nc.dram_tensor("sew1g", [C, SE_HID], f32, kind="ExternalInput")
    seb1 = nc.dram_tensor("seb1", [SE_HID, 1], f32, kind="ExternalInput")
    sew2 = nc.dram_tensor("sew2", [SE_HID, C], f32, kind="ExternalInput")
    seb2 = nc.dram_tensor("seb2", [C, 1], f32, kind="ExternalInput")
    cw_cat = nc.dram_tensor("cw_cat", [HALF, C], f32, kind="ExternalInput")
    cb_cat = nc.dram_tensor("cb_cat", [HALF, 2], f32, kind="ExternalInput")

    out_lst = nc.dram_tensor("out_lst", [C, free], f32, kind="ExternalOutput")
    out_gui = nc.dram_tensor("out_gui", [C, free], f32, kind="ExternalOutput")
    out_m = nc.dram_tensor("out_m", [1, C], f32, kind="ExternalOutput")


    with (
        nc.semaphore("rdma_rsem") as rsem,
        nc.semaphore("rdma_lsem") as lsem,
        nc.semaphore("rdma_psem") as psem,
        tile_mod.TileContext(nc) as tc,
    ):
        with (
            tc.tile_pool(name="const", bufs=1) as const,
            tc.tile_pool(name="small", bufs=1) as small,
            tc.tile_pool(name="stash", bufs=1) as stash,
            tc.tile_pool(name="io_in", bufs=4) as io_in,
            tc.tile_pool(name="b_out", bufs=2) as b_out,
            tc.tile_pool(name="ps_small", bufs=1, space="PSUM") as ps_small,
            tc.tile_pool(name="ps_big", bufs=3, space="PSUM") as ps_big,
        ):
            # ---- constants / weights in SBUF ----
            def load_const(handle, shape):
                t = const.tile(shape, f32, tag=f"c_{handle.name}")
                nc.sync.dma_start(t, handle[:])
                return t

            w1_sb = load_const(w1, [MD, C])
            b1_sb = load_const(b1, [C, 1])
            w2_sb = load_const(w2, [C, C])
            b2_sb = load_const(b2, [C, 1])
            sew1l_sb = load_const(sew1l, [C, SE_HID])
            sew1g_sb = load_const(sew1g, [C, SE_HID])
            seb1_sb = load_const(seb1, [SE_HID, 1])
            sew2_sb = load_const(sew2, [SE_HID, C])
            seb2_sb = load_const(seb2, [C, 1])
            cw_sb = load_const(cw_cat, [HALF, C])
            cb_sb = load_const(cb_cat, [HALF, 2])
            maskv_sb = load_const(maskv, [MD, 1])

            # 128x128 fp32 identity (for PE transposes)
            ident = const.tile([C, C], f32)
            nc.vector.memset(ident, 1.0)
            nc.gpsimd.affine_select(
                ident, ident, pattern=[[1, C]], base=0, channel_multiplier=-1,
                compare_op=AluOpType.is_equal, fill=0.0)

            # iota row values 0..HALF-1 per partition (fp32, exact)
            iota_j = const.tile([C, HALF], f32)
            nc.gpsimd.iota(iota_j, pattern=[[1, HALF]], base=0,
                           channel_multiplier=0,
                           allow_small_or_imprecise_dtypes=True)

            zeros_row = const.tile([1, C], f32)
            nc.vector.memset(zeros_row, 0.0)
            ones_row = const.tile([1, C], f32)
            nc.vector.memset(ones_row, 1.0)

            # ---- Phase A: channel sums of both slabs (GpSimd reduces) ----
            red_l = small.tile([C, nt], f32)
            red_g = small.tile([C, nt], f32)
            stash_tiles = {}
            for t in range(nt):
                if t >= nt - n_stash:
                    tl = stash.tile([C, tile], f32r, tag=f"st_l{t}")
                    tg = stash.tile([C, tile], f32r, tag=f"st_g{t}")
                    stash_tiles[t] = (tl, tg)
                else:
                    tl = io_in.tile([C, tile], f32r, tag="io_lst")
                    tg = io_in.tile([C, tile], f32r, tag="io_gui")
                nc.sync.dma_start(tl, lst[:, t * tile:(t + 1) * tile])
                nc.vector.tensor_reduce(red_l[:, t:t + 1], tl,
                                        AxisListType.X, AluOpType.add)
                last_a_load = nc.sync.dma_start(gui[:, t * tile:(t + 1) * tile]
                                                if False else
                                                tg, gui[:, t * tile:(t + 1) * tile])
                nc.vector.tensor_reduce(red_g[:, t:t + 1], tg,
                                        AxisListType.X, AluOpType.add)

            sums_sb = small.tile([C, 2], f32)
            nc.vector.tensor_reduce(sums_sb[:, 0:1], red_l,
                                    AxisListType.X, AluOpType.add)
            nc.vector.tensor_reduce(sums_sb[:, 1:2], red_g,
                                    AxisListType.X, AluOpType.add)

            # all-to-all of the [128, 2] partial sums via remote_dma
            # broadcast: every core lands its sums at its rank slot of
            # `land` on all 8 cores. SPMD-symmetric (relative dests, same
            # SBUF address on every core).
            mysums = small.tile([C, 2], f32)
            land = small.tile([C, 2 * ncores], f32)
            land2 = small.tile([C, 2 * ncores], f32)
            if not use_remote:
                # sim-only fallback: no cross-core exchange; slot 0 = own
                # sums, other slots zero (np reference mimics this).
                nc.vector.memset(land2, 0.0)
                nc.vector.tensor_copy(land2[:, 0:2], sums_sb[:])
            elif True:
              with tc.tile_critical():
                g = nc.gpsimd
                g.tensor_copy(mysums[:], sums_sb[:])
                g.bir_kernel_barrier_wait([list(range(ncores))])
                pid = g.partition_id()
                g.remote_dma_broadcast(
                    out_ap=land[:, bass.ds(pid * 2, 2)],
                    in_ap=mysums[:],
                    remote_sem=rsem, local_sem=lsem,
                    rdests=[(0, k) for k in range(ncores)],
                ).then_inc(psem, 1)
                g.wait_ge(psem, 1)
                g.trigger_dma(count=1)
                g.wait_ge(rsem, 16)
                g.wait_ge(lsem, 16)
                g.tensor_copy(land2[:], land[:])
            # (the 1/(ncores*free) mean scale is folded into sew1l/sew1g
            # on the host, so the raw sums feed the SE matmul directly)
            mean_sb = small.tile([C, 2, 1], f32)
            nc.vector.tensor_reduce(
                mean_sb, land2[:].rearrange("p (r c) -> p c r", c=2),
                AxisListType.X, AluOpType.add)

            # ---- mask pipeline (tiny) ----
            # FCNet: m1 = sigmoid(w2^T relu(w1^T mask + b1) + b2), column form
            ps1 = ps_small.tile([C, 1], f32, tag="ps1")
            nc.tensor.matmul(ps1, w1_sb, maskv_sb, start=True, stop=True)
            h1 = small.tile([C, 1], f32)
            nc.vector.tensor_scalar(h1, ps1, b1_sb, 0.0,
                                    AluOpType.add, AluOpType.max)
            ps2 = ps_small.tile([C, 1], f32, tag="ps1")
            nc.tensor.matmul(ps2, w2_sb, h1, start=True, stop=True)
            m1 = small.tile([C, 1], f32)
            nc.scalar.activation(m1, ps2, AF.Sigmoid, bias=b2_sb)

            # SE: m2 = sigmoid(se_w2^T relu(se_w1^T pooled + se_b1) + se_b2)
            ps3 = ps_small.tile([SE_HID, 1], f32, tag="ps1")
            nc.tensor.matmul(ps3, sew1l_sb, mean_sb[:, 0, :], start=True,
                             stop=False)
            nc.tensor.matmul(ps3, sew1g_sb, mean_sb[:, 1, :], start=False,
                             stop=True)
            hse = small.tile([SE_HID, 1], f32)
            nc.vector.tensor_scalar(hse, ps3, seb1_sb, 0.0,
                                    AluOpType.add, AluOpType.max)
            ps4 = ps_small.tile([C, 1], f32, tag="ps1")
            nc.tensor.matmul(ps4, sew2_sb, hse, start=True, stop=True)
            m2 = small.tile([C, 1], f32)
            nc.scalar.activation(m2, ps4, AF.Sigmoid, bias=seb2_sb)

            m_col = small.tile([C, 1], f32)
            nc.vector.tensor_tensor(m_col, m1, m2, AluOpType.mult)

            # transpose m -> row layout (also the out_m output)
            ps_mrow = ps_small.tile([1, C], f32, tag="ps1")
            nc.tensor.transpose(ps_mrow, m_col, ident)
            m_row = small.tile([1, C], f32)
            nc.vector.tensor_copy(m_row, ps_mrow)
            nc.sync.dma_start(out_m[:], m_row)

            # selection: cnt[c] = #{c' : m[c'] > m[c]};  s = (cnt < 64)
            ps_bc = ps_small.tile([C, C], f32, tag="ps_bc")
            nc.tensor.matmul(ps_bc, ones_row, m_row, start=True, stop=True)
            gt = small.tile([C, C], f32)
            nc.vector.tensor_scalar(gt, ps_bc, m_col, None, AluOpType.is_gt)
            cnt_col = small.tile([C, 1], f32)
            nc.vector.tensor_reduce(cnt_col, gt, AxisListType.X,
                                    AluOpType.add)
            s_col = small.tile([C, 1], f32)
            nc.vector.tensor_scalar(s_col, cnt_col, float(HALF), None,
                                    AluOpType.is_lt)
            oms = small.tile([C, 1], f32)   # 1 - s
            nc.vector.tensor_scalar(oms, s_col, -1.0, 1.0,
                                    AluOpType.mult, AluOpType.add)

            # ranks: exclusive prefix sum of s along channel index
            ps_srow = ps_small.tile([1, C], f32, tag="ps1")
            nc.tensor.transpose(ps_srow, s_col, ident)
            s_row = small.tile([1, C], f32)
            nc.vector.tensor_copy(s_row, ps_srow)
            incl = small.tile([1, C], f32)
            nc.vector.tensor_tensor_scan(incl, s_row, zeros_row, 0.0,
                                         AluOpType.add, AluOpType.add)
            rank_row = small.tile([1, C], f32)
            nc.vector.tensor_sub(rank_row, incl, s_row)
            ps_rcol = ps_small.tile([C, 1], f32, tag="ps1")
            nc.tensor.transpose(ps_rcol, rank_row, ident[:1, :1])
            rank_col = small.tile([C, 1], f32)
            nc.vector.tensor_copy(rank_col, ps_rcol)

            # selection matrix P[c, j] = s[c] * (rank[c] == j)  -> and P^T
            P = small.tile([C, HALF], f32)
            nc.vector.tensor_scalar(P, iota_j, rank_col, s_col,
                                    AluOpType.is_equal, AluOpType.mult)
            ps_pt = ps_small.tile([HALF, C], f32, tag="ps1")
            nc.tensor.transpose(ps_pt, P, ident)
            PT = small.tile([HALF, C], f32)
            nc.vector.tensor_copy(PT, ps_pt)

            # Z2 = conv2_w^T P^T ; Z1 = conv1_w^T P^T  (cw_sb = [c2w | c1w])
            ps_z2 = ps_small.tile([HALF, C], f32, tag="ps1")
            nc.tensor.matmul(ps_z2, cw_sb[:, 0:HALF], PT, start=True,
                             stop=True)
            z2_sb = small.tile([HALF, C], f32)
            nc.vector.tensor_copy(z2_sb, ps_z2)
            ps_z1 = ps_small.tile([HALF, C], f32, tag="ps1")
            nc.tensor.matmul(ps_z1, cw_sb[:, HALF:C], PT, start=True,
                             stop=True)
            z1_sb = small.tile([HALF, C], f32)
            nc.vector.tensor_copy(z1_sb, ps_z1)
            ps_a2 = ps_small.tile([C, C], f32, tag="ps_bc")
            nc.tensor.matmul(ps_a2, PT, z2_sb, start=True, stop=True)
            A2T = const.tile([C, C], f32r, tag="A2T")
            nc.vector.tensor_copy(A2T, ps_a2)
            ps_a1 = ps_small.tile([C, C], f32, tag="ps_bc")
            nc.tensor.matmul(ps_a1, PT, z1_sb, start=True, stop=True)
            A1T = const.tile([C, C], f32r, tag="A1T")
            nc.vector.tensor_copy(A1T, ps_a1)

            ps_beta = ps_small.tile([C, 2], f32, tag="ps1")
            nc.tensor.matmul(ps_beta, PT, cb_sb, start=True, stop=True)
            beta = small.tile([C, 2], f32)
            nc.vector.tensor_copy(beta, ps_beta)
            beta2 = beta[:, 0:1]
            beta1 = beta[:, 1:2]

            # ---- Phase B: streamed exchange (stashed tiles first) ----
            from concourse.tile import add_dep_helper

            order = list(range(nt - n_stash, nt)) + list(range(nt - n_stash))
            n_deferred = 0
            for t in order:
                if t in stash_tiles:
                    li, gi = stash_tiles[t]
                else:
                    li = io_in.tile([C, tile], f32r, tag="io_lst")
                    d1 = nc.sync.dma_start(li, lst[:, t * tile:(t + 1) * tile])
                    gi = io_in.tile([C, tile], f32r, tag="io_gui")
                    d2 = nc.sync.dma_start(gi, gui[:, t * tile:(t + 1) * tile])
                    if n_deferred < 4:
                        # keep phase-A loads ahead of the re-stream prefetch
                        # so the sums (and the exchange) trigger early
                        add_dep_helper(d1.ins, last_a_load.ins, sync=False,
                                       reason="defer phase-B prefetch")
                        add_dep_helper(d2.ins, last_a_load.ins, sync=False,
                                       reason="defer phase-B prefetch")
                        n_deferred += 1
                lo = b_out.tile([C, tile], f32, tag="o_lst")
                go = b_out.tile([C, tile], f32, tag="o_gui")
                for j in range(mm_per_tile):
                    sl = slice(j * mm, (j + 1) * mm)
                    pl = ps_big.tile([C, mm], f32, tag="ps_l")
                    nc.tensor.matmul(pl, A2T[:], gi[:, sl],
                                     start=True, stop=True)
                    nc.scalar.activation(lo[:, sl], li[:, sl], AF.Identity,
                                         bias=beta2, scale=oms)
                    nc.vector.tensor_add(lo[:, sl], lo[:, sl], pl)

                    pg = ps_big.tile([C, mm], f32, tag="ps_g")
                    nc.tensor.matmul(pg, A1T[:], li[:, sl],
                                     start=True, stop=True)
                    nc.scalar.activation(go[:, sl], gi[:, sl], AF.Identity,
                                         bias=beta1, scale=oms)
                    nc.vector.tensor_add(go[:, sl], go[:, sl], pg)
                nc.sync.dma_start(out_lst[:, t * tile:(t + 1) * tile], lo)
                nc.sync.dma_start(out_gui[:, t * tile:(t + 1) * tile], go)

    nc.compile()
    return nc


def make_in_maps(lst, gui, mask, w1, b1, w2, b2, se_w1, se_b1, se_w2, se_b2,
                 conv1_w, conv1_b, conv2_w, conv2_b, ncores=NCORES,
                 hshard=HSHARD, mean_scale=None):
    """Shard the full inputs into per-core input maps."""
    f = np.float32
    if mean_scale is None:
        mean_scale = 1.0 / (ncores * hshard * W)
    lst = np.asarray(lst, f)
    gui = np.asarray(gui, f)
    cw = np.concatenate([np.asarray(conv2_w, f), np.asarray(conv1_w, f)],
                        axis=1)
    cb = np.stack([np.asarray(conv2_b, f), np.asarray(conv1_b, f)], axis=1)
    shared = {
        "maskv": np.ascontiguousarray(np.asarray(mask, f).reshape(1, MD).T),
        "w1": np.ascontiguousarray(np.asarray(w1, f)),
        "b1": np.ascontiguousarray(np.asarray(b1, f).reshape(C, 1)),
        "w2": np.ascontiguousarray(np.asarray(w2, f)),
        "b2": np.ascontiguousarray(np.asarray(b2, f).reshape(C, 1)),
        "sew1l": np.ascontiguousarray(np.asarray(se_w1, f)[:C] * f(mean_scale)),
        "sew1g": np.ascontiguousarray(np.asarray(se_w1, f)[C:] * f(mean_scale)),
        "seb1": np.ascontiguousarray(np.asarray(se_b1, f).reshape(SE_HID, 1)),
        "sew2": np.ascontiguousarray(np.asarray(se_w2, f)),
        "seb2": np.ascontiguousarray(np.asarray(se_b2, f).reshape(C, 1)),
        "cw_cat": np.ascontiguousarray(cw),
        "cb_cat": np.ascontiguousarray(cb),
    }
    in_maps = []
    for i in range(ncores):
        sl = slice(i * hshard, (i + 1) * hshard)
        in_maps.append({
            "lst": np.ascontiguousarray(lst[0, :, sl, :]).reshape(C, -1),
            "gui": np.ascontiguousarray(gui[0, :, sl, :]).reshape(C, -1),
            **shared,
        })
    return in_maps


_NC_CACHE = {}


def kernel(lst, gui, mask, w1, b1, w2, b2, se_w1, se_b1, se_w2, se_b2,
           conv1_w, conv1_b, conv2_w, conv2_b):
    from concourse.bass_utils import run_bass_kernel_spmd

    if "nc" not in _NC_CACHE:
        _NC_CACHE["nc"] = build_nc()
    nc = _NC_CACHE["nc"]

    in_maps = make_in_maps(lst, gui, mask, w1, b1, w2, b2, se_w1, se_b1,
                           se_w2, se_b2, conv1_w, conv1_b, conv2_w, conv2_b)
    res = run_bass_kernel_spmd(nc, in_maps, core_ids=list(range(NCORES)))

    out_lst = np.empty((1, C, H, W), np.float32)
    out_gui = np.empty((1, C, H, W), np.float32)
    for i in range(NCORES):
        sl = slice(i * HSHARD, (i + 1) * HSHARD)
        out_lst[0, :, sl, :] = res.results[i]["out_lst"].reshape(C, HSHARD, W)
        out_gui[0, :, sl, :] = res.results[i]["out_gui"].reshape(C, HSHARD, W)
    m = res.results[0]["out_m"].reshape(1, C).copy()
    return out_lst, out_gui, m


# revision 16
# speedup vs baseline: 1.0449x; 1.0449x over previous
"""Trainium2 Bass kernel for DynamicChannelExchangeWithSE.

Contract: kernel(**inputs) takes the FULL unsharded inputs (numpy, keyed as in
setup_inputs) and returns the full (out_lst, out_gui, m) tuple.

Strategy (8 NeuronCores, H sharded 8-ways -> per-core [128, 64*512] slabs):
  Phase A  stream both slabs once; GpSimd accumulates per-channel partial
           sums. The last STASH tiles per slab stay resident in SBUF.
  AllGather the [128, 2] partial sums across the 8 cores (1 KB payload),
           combine on DVE.
  Mask     tiny on-device pipeline: FCNet MLP + SE MLP (PE matmuls, DVE
           relu, ACT sigmoid), top-64 selection by rank counting (PE
           outer-product broadcast + DVE compare/reduce), ranks via prefix
           scan, selection matrix P, then A2T = P conv2_w^T P^T etc.
  Phase B  process stashed tiles first (no DMA), then re-stream the rest;
           per 512-col chunk:
              out_lst = (lst*(1-s) + beta2) + A2T.T @ gui
              out_gui = (gui*(1-s) + beta1) + A1T.T @ lst
           (PE fp32 matmul into PSUM; DVE tensor_scalar folds the
           passthrough+bias; DVE tensor_tensor adds the PSUM.)
"""

import numpy as np

C = 128          # channels per tensor
HALF = 64        # selected channel count / conv dims
MD = 64          # mask input dim
SE_HID = 16
H = 512
W = 512
NCORES = 8
HSHARD = H // NCORES          # 64 rows per core
FREE = HSHARD * W             # 32768 columns per core


def build_nc(free=FREE, tile=2048, mm=512, ncores=NCORES, n_stash=6,
             use_f32r=True, sync_mode='remote', io_bufs=4):
    """Build the per-core Bass program. All cores run the identical program."""
    import concourse.bacc as bacc
    import concourse.bass as bass
    import concourse.mybir as mybir
    from concourse import tile as tile_mod
    from concourse.alu_op_type import AluOpType
    from bass_rust import AxisListType

    f32 = mybir.dt.float32
    f32r = mybir.dt.float32r if use_f32r else mybir.dt.float32
    AF = mybir.ActivationFunctionType
    nt = free // tile
    assert tile % mm == 0 and free % tile == 0
    assert 0 <= n_stash < nt
    mm_per_tile = tile // mm

    nc = bacc.Bacc("TRN2", target_bir_lowering=False, debug=False,
                   num_devices=ncores)

    lst = nc.dram_tensor("lst", [C, free], f32r, kind="ExternalInput")
    gui = nc.dram_tensor("gui", [C, free], f32r, kind="ExternalInput")
    maskv = nc.dram_tensor("maskv", [MD, 1], f32, kind="ExternalInput")
    w1 = nc.dram_tensor("w1", [MD, C], f32, kind="ExternalInput")
    b1 = nc.dram_tensor("b1", [C, 1], f32, kind="ExternalInput")
    w2 = nc.dram_tensor("w2", [C, C], f32, kind="ExternalInput")
    b2 = nc.dram_tensor("b2", [C, 1], f32, kind="ExternalInput")
    sew1l = nc.dram_tensor("sew1l", [C, SE_HID], f32, kind="ExternalInput")
    sew1g = nc.dram_tensor("sew1g", [C, SE_HID], f32, kind="ExternalInput")
    seb1 = nc.dram_tensor("seb1", [SE_HID, 1], f32, kind="ExternalInput")
    sew2 = nc.dram_tensor("sew2", [SE_HID, C], f32, kind="ExternalInput")
    seb2 = nc.dram_tensor("seb2", [C, 1], f32, kind="ExternalInput")
    cw_cat = nc.dram_tensor("cw_cat", [HALF, C], f32, kind="ExternalInput")
    cb_cat = nc.dram_tensor("cb_cat", [HALF, 2], f32, kind="ExternalInput")

    cc_in = nc.dram_tensor("cc_in", [C, 2], f32)
    cc_out = nc.dram_tensor("cc_out", [C * ncores, 2], f32,
                            addr_space="Shared")
    ccw_in = nc.dram_tensor("ccw_in", [1, 8], f32)
    ccw_out = nc.dram_tensor("ccw_out", [ncores, 8], f32,
                             addr_space="Shared")

    out_lst = nc.dram_tensor("out_lst", [C, free], f32, kind="ExternalOutput")
    out_gui = nc.dram_tensor("out_gui", [C, free], f32, kind="ExternalOutput")
    out_m = nc.dram_tensor("out_m", [1, C], f32, kind="ExternalOutput")


    with (
        nc.semaphore("rdma_rsem") as rsem,
        nc.semaphore("rdma_lsem") as lsem,
        nc.semaphore("rdma_psem") as psem,
        tile_mod.TileContext(nc) as tc,
    ):
        with (
            tc.tile_pool(name="const", bufs=1) as const,
            tc.tile_pool(name="small", bufs=1) as small,
            tc.tile_pool(name="stash", bufs=1) as stash,
            tc.tile_pool(name="io_in", bufs=io_bufs) as io_in,
            tc.tile_pool(name="b_out", bufs=6) as b_out,
            tc.tile_pool(name="ps_small", bufs=1, space="PSUM") as ps_small,
            tc.tile_pool(name="ps_big", bufs=3, space="PSUM") as ps_big,
        ):
            # ---- constants / weights in SBUF ----
            def load_const(handle, shape):
                t = const.tile(shape, f32, tag=f"c_{handle.name}")
                nc.sync.dma_start(t, handle[:])
                return t

            w1_sb = load_const(w1, [MD, C])
            b1_sb = load_const(b1, [C, 1])
            w2_sb = load_const(w2, [C, C])
            b2_sb = load_const(b2, [C, 1])
            sew1l_sb = load_const(sew1l, [C, SE_HID])
            sew1g_sb = load_const(sew1g, [C, SE_HID])
            seb1_sb = load_const(seb1, [SE_HID, 1])
            sew2_sb = load_const(sew2, [SE_HID, C])
            seb2_sb = load_const(seb2, [C, 1])
            cw_sb = load_const(cw_cat, [HALF, C])
            cb_sb = load_const(cb_cat, [HALF, 2])
            maskv_sb = load_const(maskv, [MD, 1])

            # 128x128 fp32 identity (for PE transposes)
            ident = const.tile([C, C], f32)
            nc.vector.memset(ident, 1.0)
            nc.gpsimd.affine_select(
                ident, ident, pattern=[[1, C]], base=0, channel_multiplier=-1,
                compare_op=AluOpType.is_equal, fill=0.0)

            # iota row values 0..HALF-1 per partition (fp32, exact)
            iota_j = const.tile([C, HALF], f32)
            nc.gpsimd.iota(iota_j, pattern=[[1, HALF]], base=0,
                           channel_multiplier=0,
                           allow_small_or_imprecise_dtypes=True)

            zeros_row = const.tile([1, C], f32)
            nc.vector.memset(zeros_row, 0.0)
            ones_row = const.tile([1, C], f32)
            nc.vector.memset(ones_row, 1.0)

            if sync_mode == 'ag':
                warm = small.tile([1, 8], f32)
                nc.vector.memset(warm, 0.0)
                nc.sync.dma_start(ccw_in[:], warm)
                nc.gpsimd.collective_compute(
                    "AllGather", AluOpType.bypass,
                    replica_groups=[list(range(ncores))],
                    ins=[ccw_in[:]], outs=[ccw_out[:]])

            # ---- Phase A: channel sums of both slabs ----
            red_l = small.tile([C, nt], f32)
            red_g = small.tile([C, nt], f32)
            stash_tiles = {}
            for t in range(nt):
                if t >= nt - n_stash:
                    tl = stash.tile([C, tile], f32r, tag=f"st_l{t}")
                    tg = stash.tile([C, tile], f32r, tag=f"st_g{t}")
                    stash_tiles[t] = (tl, tg)
                else:
                    tl = io_in.tile([C, tile], f32r, tag="io_lst")
                    tg = io_in.tile([C, tile], f32r, tag="io_gui")
                nc.sync.dma_start(tl, lst[:, t * tile:(t + 1) * tile])
                nc.vector.tensor_reduce(red_l[:, t:t + 1], tl,
                                        AxisListType.X, AluOpType.add)
                last_a_load = nc.sync.dma_start(gui[:, t * tile:(t + 1) * tile]
                                                if False else
                                                tg, gui[:, t * tile:(t + 1) * tile])
                nc.vector.tensor_reduce(red_g[:, t:t + 1], tg,
                                        AxisListType.X, AluOpType.add)

            sums_sb = small.tile([C, 2], f32)
            nc.vector.tensor_reduce(sums_sb[:, 0:1], red_l,
                                    AxisListType.X, AluOpType.add)
            nc.vector.tensor_reduce(sums_sb[:, 1:2], red_g,
                                    AxisListType.X, AluOpType.add)

            # all-to-all of the [128, 2] partial sums via remote_dma
            # broadcast: every core lands its sums at its rank slot of
            # `land` on all 8 cores. SPMD-symmetric (relative dests, same
            # SBUF address on every core).
            mysums = small.tile([C, 2], f32)
            land = small.tile([C, 2 * ncores], f32)
            land2 = small.tile([C, 2 * ncores], f32)
            if sync_mode == 'local':
                # sim-only fallback: no cross-core exchange; slot 0 = own
                # sums, other slots zero (np reference mimics this).
                nc.vector.memset(land2, 0.0)
                nc.vector.tensor_copy(land2[:, 0:2], sums_sb[:])
            elif sync_mode == 'ag':
                nc.sync.dma_start(cc_in[:], sums_sb)
                nc.gpsimd.collective_compute(
                    "AllGather", AluOpType.bypass,
                    replica_groups=[list(range(ncores))],
                    ins=[cc_in[:]], outs=[cc_out[:]])
                nc.sync.dma_start(
                    land2[:].rearrange("p (r c) -> p r c", c=2),
                    cc_out[:].rearrange("(r p) c -> p r c", r=ncores))
            else:
              with tc.tile_critical():
                g = nc.gpsimd
                g.tensor_copy(mysums[:], sums_sb[:])
                g.bir_kernel_barrier_wait([list(range(ncores))])
                pid = g.partition_id()
                g.remote_dma_broadcast(
                    out_ap=land[:, bass.ds(pid * 2, 2)],
                    in_ap=mysums[:],
                    remote_sem=rsem, local_sem=lsem,
                    rdests=[(0, k) for k in range(ncores)],
                ).then_inc(psem, 1)
                g.wait_ge(psem, 1)
                g.trigger_dma(count=1)
                g.wait_ge(rsem, 16)
                g.wait_ge(lsem, 16)
                g.tensor_copy(land2[:], land[:])
            # (the 1/(ncores*free) mean scale is folded into sew1l/sew1g
            # on the host, so the raw sums feed the SE matmul directly)
            mean_sb = small.tile([C, 2, 1], f32)
            nc.vector.tensor_reduce(
                mean_sb, land2[:].rearrange("p (r c) -> p c r", c=2),
                AxisListType.X, AluOpType.add)

            # ---- mask pipeline (tiny) ----
            # FCNet: m1 = sigmoid(w2^T relu(w1^T mask + b1) + b2), column form
            ps1 = ps_small.tile([C, 1], f32, tag="ps1")
            nc.tensor.matmul(ps1, w1_sb, maskv_sb, start=True, stop=True)
            h1 = small.tile([C, 1], f32)
            nc.vector.tensor_scalar(h1, ps1, b1_sb, 0.0,
                                    AluOpType.add, AluOpType.max)
            ps2 = ps_small.tile([C, 1], f32, tag="ps1")
            nc.tensor.matmul(ps2, w2_sb, h1, start=True, stop=True)
            m1 = small.tile([C, 1], f32)
            nc.scalar.activation(m1, ps2, AF.Sigmoid, bias=b2_sb)

            # SE: m2 = sigmoid(se_w2^T relu(se_w1^T pooled + se_b1) + se_b2)
            ps3 = ps_small.tile([SE_HID, 1], f32, tag="ps1")
            nc.tensor.matmul(ps3, sew1l_sb, mean_sb[:, 0, :], start=True,
                             stop=False)
            nc.tensor.matmul(ps3, sew1g_sb, mean_sb[:, 1, :], start=False,
                             stop=True)
            hse = small.tile([SE_HID, 1], f32)
            nc.vector.tensor_scalar(hse, ps3, seb1_sb, 0.0,
                                    AluOpType.add, AluOpType.max)
            ps4 = ps_small.tile([C, 1], f32, tag="ps1")
            nc.tensor.matmul(ps4, sew2_sb, hse, start=True, stop=True)
            m2 = small.tile([C, 1], f32)
            nc.scalar.activation(m2, ps4, AF.Sigmoid, bias=seb2_sb)

            m_col = small.tile([C, 1], f32)
            nc.vector.tensor_tensor(m_col, m1, m2, AluOpType.mult)

            # transpose m -> row layout (also the out_m output)
            ps_mrow = ps_small.tile([1, C], f32, tag="ps1")
            nc.tensor.transpose(ps_mrow, m_col, ident)
            m_row = small.tile([1, C], f32)
            nc.vector.tensor_copy(m_row, ps_mrow)
            nc.sync.dma_start(out_m[:], m_row)

            # selection: cnt[c] = #{c' : m[c'] > m[c]};  s = (cnt < 64)
            ps_bc = ps_small.tile([C, C], f32, tag="ps_bc")
            nc.tensor.matmul(ps_bc, ones_row, m_row, start=True, stop=True)
            gt = small.tile([C, C], f32)
            nc.vector.tensor_scalar(gt, ps_bc, m_col, None, AluOpType.is_gt)
            cnt_col = small.tile([C, 1], f32)
            nc.vector.tensor_reduce(cnt_col, gt, AxisListType.X,
                                    AluOpType.add)
            s_col = small.tile([C, 1], f32)
            nc.vector.tensor_scalar(s_col, cnt_col, float(HALF), None,
                                    AluOpType.is_lt)
            oms = small.tile([C, 1], f32)   # 1 - s
            nc.vector.tensor_scalar(oms, s_col, -1.0, 1.0,
                                    AluOpType.mult, AluOpType.add)

            # ranks: exclusive prefix sum of s along channel index
            ps_srow = ps_small.tile([1, C], f32, tag="ps1")
            nc.tensor.transpose(ps_srow, s_col, ident)
            s_row = small.tile([1, C], f32)
            nc.vector.tensor_copy(s_row, ps_srow)
            incl = small.tile([1, C], f32)
            nc.vector.tensor_tensor_scan(incl, s_row, zeros_row, 0.0,
                                         AluOpType.add, AluOpType.add)
            rank_row = small.tile([1, C], f32)
            nc.vector.tensor_sub(rank_row, incl, s_row)
            ps_rcol = ps_small.tile([C, 1], f32, tag="ps1")
            nc.tensor.transpose(ps_rcol, rank_row, ident[:1, :1])
            rank_col = small.tile([C, 1], f32)
            nc.vector.tensor_copy(rank_col, ps_rcol)

            # selection matrix P[c, j] = s[c] * (rank[c] == j)  -> and P^T
            P = small.tile([C, HALF], f32)
            nc.vector.tensor_scalar(P, iota_j, rank_col, s_col,
                                    AluOpType.is_equal, AluOpType.mult)
            ps_pt = ps_small.tile([HALF, C], f32, tag="ps1")
            nc.tensor.transpose(ps_pt, P, ident)
            PT = small.tile([HALF, C], f32)
            nc.vector.tensor_copy(PT, ps_pt)

            # Z2 = conv2_w^T P^T ; Z1 = conv1_w^T P^T  (cw_sb = [c2w | c1w])
            ps_z2 = ps_small.tile([HALF, C], f32, tag="ps1")
            nc.tensor.matmul(ps_z2, cw_sb[:, 0:HALF], PT, start=True,
                             stop=True)
            z2_sb = small.tile([HALF, C], f32)
            nc.vector.tensor_copy(z2_sb, ps_z2)
            ps_z1 = ps_small.tile([HALF, C], f32, tag="ps1")
            nc.tensor.matmul(ps_z1, cw_sb[:, HALF:C], PT, start=True,
                             stop=True)
            z1_sb = small.tile([HALF, C], f32)
            nc.vector.tensor_copy(z1_sb, ps_z1)
            ps_a2 = ps_small.tile([C, C], f32, tag="ps_bc")
            nc.tensor.matmul(ps_a2, PT, z2_sb, start=True, stop=True)
            A2T = const.tile([C, C], f32r, tag="A2T")
            nc.vector.tensor_copy(A2T, ps_a2)
            ps_a1 = ps_small.tile([C, C], f32, tag="ps_bc")
            nc.tensor.matmul(ps_a1, PT, z1_sb, start=True, stop=True)
            A1T = const.tile([C, C], f32r, tag="A1T")
            nc.vector.tensor_copy(A1T, ps_a1)

            ps_beta = ps_small.tile([C, 2], f32, tag="ps1")
            nc.tensor.matmul(ps_beta, PT, cb_sb, start=True, stop=True)
            beta = small.tile([C, 2], f32)
            nc.vector.tensor_copy(beta, ps_beta)
            beta2 = beta[:, 0:1]
            beta1 = beta[:, 1:2]

            # ---- Phase B: streamed exchange (stashed tiles first) ----
            from concourse.tile import add_dep_helper

            order = list(range(nt - n_stash, nt)) + list(range(nt - n_stash))
            n_deferred = 0
            for t in order:
                if t in stash_tiles:
                    li, gi = stash_tiles[t]
                else:
                    li = io_in.tile([C, tile], f32r, tag="io_lst")
                    d1 = nc.sync.dma_start(li, lst[:, t * tile:(t + 1) * tile])
                    gi = io_in.tile([C, tile], f32r, tag="io_gui")
                    d2 = nc.sync.dma_start(gi, gui[:, t * tile:(t + 1) * tile])
                    if n_deferred < 4:
                        # keep phase-A loads ahead of the re-stream prefetch
                        # so the sums (and the exchange) trigger early
                        add_dep_helper(d1.ins, last_a_load.ins, sync=False,
                                       reason="defer phase-B prefetch")
                        add_dep_helper(d2.ins, last_a_load.ins, sync=False,
                                       reason="defer phase-B prefetch")
                        n_deferred += 1
                for j in range(mm_per_tile):
                    sl = slice(j * mm, (j + 1) * mm)
                    c0 = t * tile + j * mm
                    pl = ps_big.tile([C, mm], f32, tag="ps_l")
                    nc.tensor.matmul(pl, A2T[:], gi[:, sl],
                                     start=True, stop=True)
                    lo = b_out.tile([C, mm], f32, tag="o_lst")
                    nc.scalar.activation(lo, li[:, sl], AF.Identity,
                                         bias=beta2, scale=oms)
                    nc.vector.tensor_add(lo, lo, pl)
                    nc.sync.dma_start(out_lst[:, c0:c0 + mm], lo)

                    pg = ps_big.tile([C, mm], f32, tag="ps_g")
                    nc.tensor.matmul(pg, A1T[:], li[:, sl],
                                     start=True, stop=True)
                    go = b_out.tile([C, mm], f32, tag="o_gui")
                    nc.scalar.activation(go, gi[:, sl], AF.Identity,
                                         bias=beta1, scale=oms)
                    nc.vector.tensor_add(go, go, pg)
                    nc.sync.dma_start(out_gui[:, c0:c0 + mm], go)

    nc.compile()
    return nc


def make_in_maps(lst, gui, mask, w1, b1, w2, b2, se_w1, se_b1, se_w2, se_b2,
                 conv1_w, conv1_b, conv2_w, conv2_b, ncores=NCORES,
                 hshard=HSHARD, mean_scale=None):
    """Shard the full inputs into per-core input maps."""
    f = np.float32
    if mean_scale is None:
        mean_scale = 1.0 / (ncores * hshard * W)
    lst = np.asarray(lst, f)
    gui = np.asarray(gui, f)
    cw = np.concatenate([np.asarray(conv2_w, f), np.asarray(conv1_w, f)],
                        axis=1)
    cb = np.stack([np.asarray(conv2_b, f), np.asarray(conv1_b, f)], axis=1)
    shared = {
        "maskv": np.ascontiguousarray(np.asarray(mask, f).reshape(1, MD).T),
        "w1": np.ascontiguousarray(np.asarray(w1, f)),
        "b1": np.ascontiguousarray(np.asarray(b1, f).reshape(C, 1)),
        "w2": np.ascontiguousarray(np.asarray(w2, f)),
        "b2": np.ascontiguousarray(np.asarray(b2, f).reshape(C, 1)),
        "sew1l": np.ascontiguousarray(np.asarray(se_w1, f)[:C] * f(mean_scale)),
        "sew1g": np.ascontiguousarray(np.asarray(se_w1, f)[C:] * f(mean_scale)),
        "seb1": np.ascontiguousarray(np.asarray(se_b1, f).reshape(SE_HID, 1)),
        "sew2": np.ascontiguousarray(np.asarray(se_w2, f)),
        "seb2": np.ascontiguousarray(np.asarray(se_b2, f).reshape(C, 1)),
        "cw_cat": np.ascontiguousarray(cw),
        "cb_cat": np.ascontiguousarray(cb),
    }
    in_maps = []
    for i in range(ncores):
        sl = slice(i * hshard, (i + 1) * hshard)
        in_maps.append({
            "lst": np.ascontiguousarray(lst[0, :, sl, :]).reshape(C, -1),
            "gui": np.ascontiguousarray(gui[0, :, sl, :]).reshape(C, -1),
            **shared,
        })
    return in_maps


_NC_CACHE = {}


def kernel(lst, gui, mask, w1, b1, w2, b2, se_w1, se_b1, se_w2, se_b2,
           conv1_w, conv1_b, conv2_w, conv2_b):
    from concourse.bass_utils import run_bass_kernel_spmd

    if "nc" not in _NC_CACHE:
        _NC_CACHE["nc"] = build_nc()
    nc = _NC_CACHE["nc"]

    in_maps = make_in_maps(lst, gui, mask, w1, b1, w2, b2, se_w1, se_b1,
                           se_w2, se_b2, conv1_w, conv1_b, conv2_w, conv2_b)
    res = run_bass_kernel_spmd(nc, in_maps, core_ids=list(range(NCORES)))

    out_lst = np.empty((1, C, H, W), np.float32)
    out_gui = np.empty((1, C, H, W), np.float32)
    for i in range(NCORES):
        sl = slice(i * HSHARD, (i + 1) * HSHARD)
        out_lst[0, :, sl, :] = res.results[i]["out_lst"].reshape(C, HSHARD, W)
        out_gui[0, :, sl, :] = res.results[i]["out_gui"].reshape(C, HSHARD, W)
    m = res.results[0]["out_m"].reshape(1, C).copy()
    return out_lst, out_gui, m


# revision 17
# speedup vs baseline: 1.0588x; 1.0134x over previous
"""Trainium2 Bass kernel for DynamicChannelExchangeWithSE.

Contract: kernel(**inputs) takes the FULL unsharded inputs (numpy, keyed as in
setup_inputs) and returns the full (out_lst, out_gui, m) tuple.

Strategy (8 NeuronCores, H sharded 8-ways -> per-core [128, 64*512] slabs):
  Phase A  stream both slabs once; GpSimd accumulates per-channel partial
           sums. The last STASH tiles per slab stay resident in SBUF.
  AllGather the [128, 2] partial sums across the 8 cores (1 KB payload),
           combine on DVE.
  Mask     tiny on-device pipeline: FCNet MLP + SE MLP (PE matmuls, DVE
           relu, ACT sigmoid), top-64 selection by rank counting (PE
           outer-product broadcast + DVE compare/reduce), ranks via prefix
           scan, selection matrix P, then A2T = P conv2_w^T P^T etc.
  Phase B  process stashed tiles first (no DMA), then re-stream the rest;
           per 512-col chunk:
              out_lst = (lst*(1-s) + beta2) + A2T.T @ gui
              out_gui = (gui*(1-s) + beta1) + A1T.T @ lst
           (PE fp32 matmul into PSUM; ACT Identity folds passthrough+bias;
           DVE tensor_tensor adds the PSUM; chunk-granular output stores.)
"""

import numpy as np

C = 128          # channels per tensor
HALF = 64        # selected channel count / conv dims
MD = 64          # mask input dim
SE_HID = 16
H = 512
W = 512
NCORES = 8
HSHARD = H // NCORES          # 64 rows per core
FREE = HSHARD * W             # 32768 columns per core


def build_nc(free=FREE, tile=2048, mm=512, ncores=NCORES, n_stash=6,
             use_f32r=False, sync_mode='ag', io_bufs=4):
    """Build the per-core Bass program. All cores run the identical program."""
    import concourse.bacc as bacc
    import concourse.bass as bass
    import concourse.mybir as mybir
    from concourse import tile as tile_mod
    from concourse.alu_op_type import AluOpType
    from bass_rust import AxisListType

    f32 = mybir.dt.float32
    f32r = mybir.dt.float32r if use_f32r else mybir.dt.float32
    AF = mybir.ActivationFunctionType
    nt = free // tile
    assert tile % mm == 0 and free % tile == 0
    assert 0 <= n_stash < nt
    mm_per_tile = tile // mm

    nc = bacc.Bacc("TRN2", target_bir_lowering=False, debug=False,
                   num_devices=ncores)

    lst = nc.dram_tensor("lst", [C, free], f32r, kind="ExternalInput")
    gui = nc.dram_tensor("gui", [C, free], f32r, kind="ExternalInput")
    maskv = nc.dram_tensor("maskv", [MD, 1], f32, kind="ExternalInput")
    w1 = nc.dram_tensor("w1", [MD, C], f32, kind="ExternalInput")
    b1 = nc.dram_tensor("b1", [C, 1], f32, kind="ExternalInput")
    w2 = nc.dram_tensor("w2", [C, C], f32, kind="ExternalInput")
    b2 = nc.dram_tensor("b2", [C, 1], f32, kind="ExternalInput")
    sew1l = nc.dram_tensor("sew1l", [C, SE_HID], f32, kind="ExternalInput")
    sew1g = nc.dram_tensor("sew1g", [C, SE_HID], f32, kind="ExternalInput")
    seb1 = nc.dram_tensor("seb1", [SE_HID, 1], f32, kind="ExternalInput")
    sew2 = nc.dram_tensor("sew2", [SE_HID, C], f32, kind="ExternalInput")
    seb2 = nc.dram_tensor("seb2", [C, 1], f32, kind="ExternalInput")
    cw_cat = nc.dram_tensor("cw_cat", [HALF, C], f32, kind="ExternalInput")
    cb_cat = nc.dram_tensor("cb_cat", [HALF, 2], f32, kind="ExternalInput")

    cc_in = nc.dram_tensor("cc_in", [C, 2], f32)
    cc_out = nc.dram_tensor("cc_out", [C * ncores, 2], f32,
                            addr_space="Shared")
    ccw_in = nc.dram_tensor("ccw_in", [1, 8], f32)
    ccw_out = nc.dram_tensor("ccw_out", [ncores, 8], f32,
                             addr_space="Shared")

    out_lst = nc.dram_tensor("out_lst", [C, free], f32, kind="ExternalOutput")
    out_gui = nc.dram_tensor("out_gui", [C, free], f32, kind="ExternalOutput")
    out_m = nc.dram_tensor("out_m", [1, C], f32, kind="ExternalOutput")


    with (
        nc.semaphore("rdma_rsem") as rsem,
        nc.semaphore("rdma_lsem") as lsem,
        nc.semaphore("rdma_psem") as psem,
        tile_mod.TileContext(nc) as tc,
    ):
        with (
            tc.tile_pool(name="const", bufs=1) as const,
            tc.tile_pool(name="small", bufs=1) as small,
            tc.tile_pool(name="stash", bufs=1) as stash,
            tc.tile_pool(name="io_in", bufs=io_bufs) as io_in,
            tc.tile_pool(name="b_out", bufs=6) as b_out,
            tc.tile_pool(name="ps_small", bufs=1, space="PSUM") as ps_small,
            tc.tile_pool(name="ps_big", bufs=3, space="PSUM") as ps_big,
        ):
            # ---- constants / weights in SBUF ----
            def load_const(handle, shape):
                t = const.tile(shape, f32, tag=f"c_{handle.name}")
                nc.sync.dma_start(t, handle[:])
                return t

            w1_sb = load_const(w1, [MD, C])
            b1_sb = load_const(b1, [C, 1])
            w2_sb = load_const(w2, [C, C])
            b2_sb = load_const(b2, [C, 1])
            sew1l_sb = load_const(sew1l, [C, SE_HID])
            sew1g_sb = load_const(sew1g, [C, SE_HID])
            seb1_sb = load_const(seb1, [SE_HID, 1])
            sew2_sb = load_const(sew2, [SE_HID, C])
            seb2_sb = load_const(seb2, [C, 1])
            cw_sb = load_const(cw_cat, [HALF, C])
            cb_sb = load_const(cb_cat, [HALF, 2])
            maskv_sb = load_const(maskv, [MD, 1])

            # 128x128 fp32 identity (for PE transposes)
            ident = const.tile([C, C], f32)
            nc.vector.memset(ident, 1.0)
            nc.gpsimd.affine_select(
                ident, ident, pattern=[[1, C]], base=0, channel_multiplier=-1,
                compare_op=AluOpType.is_equal, fill=0.0)

            # iota row values 0..HALF-1 per partition (fp32, exact)
            iota_j = const.tile([C, HALF], f32)
            nc.gpsimd.iota(iota_j, pattern=[[1, HALF]], base=0,
                           channel_multiplier=0,
                           allow_small_or_imprecise_dtypes=True)

            zeros_row = const.tile([1, C], f32)
            nc.vector.memset(zeros_row, 0.0)
            ones_row = const.tile([1, C], f32)
            nc.vector.memset(ones_row, 1.0)

            if sync_mode == 'ag':
                warm = small.tile([1, 8], f32)
                nc.vector.memset(warm, 0.0)
                nc.sync.dma_start(ccw_in[:], warm)
                nc.gpsimd.collective_compute(
                    "AllGather", AluOpType.bypass,
                    replica_groups=[list(range(ncores))],
                    ins=[ccw_in[:]], outs=[ccw_out[:]])

            # ---- Phase A: channel sums of both slabs ----
            red_l = small.tile([C, nt], f32)
            red_g = small.tile([C, nt], f32)
            stash_tiles = {}
            for t in range(nt):
                if t >= nt - n_stash:
                    tl = stash.tile([C, tile], f32r, tag=f"st_l{t}")
                    tg = stash.tile([C, tile], f32r, tag=f"st_g{t}")
                    stash_tiles[t] = (tl, tg)
                else:
                    tl = io_in.tile([C, tile], f32r, tag="io_lst")
                    tg = io_in.tile([C, tile], f32r, tag="io_gui")
                nc.sync.dma_start(tl, lst[:, t * tile:(t + 1) * tile])
                nc.vector.tensor_reduce(red_l[:, t:t + 1], tl,
                                        AxisListType.X, AluOpType.add)
                last_a_load = nc.sync.dma_start(gui[:, t * tile:(t + 1) * tile]
                                                if False else
                                                tg, gui[:, t * tile:(t + 1) * tile])
                nc.vector.tensor_reduce(red_g[:, t:t + 1], tg,
                                        AxisListType.X, AluOpType.add)

            sums_sb = small.tile([C, 2], f32)
            nc.vector.tensor_reduce(sums_sb[:, 0:1], red_l,
                                    AxisListType.X, AluOpType.add)
            nc.vector.tensor_reduce(sums_sb[:, 1:2], red_g,
                                    AxisListType.X, AluOpType.add)

            # all-to-all of the [128, 2] partial sums via remote_dma
            # broadcast: every core lands its sums at its rank slot of
            # `land` on all 8 cores. SPMD-symmetric (relative dests, same
            # SBUF address on every core).
            mysums = small.tile([C, 2], f32)
            land = small.tile([C, 2 * ncores], f32)
            land2 = small.tile([C, 2 * ncores], f32)
            if sync_mode == 'local':
                # sim-only fallback: no cross-core exchange; slot 0 = own
                # sums, other slots zero (np reference mimics this).
                nc.vector.memset(land2, 0.0)
                nc.vector.tensor_copy(land2[:, 0:2], sums_sb[:])
            elif sync_mode == 'ag':
                nc.sync.dma_start(cc_in[:], sums_sb)
                nc.gpsimd.collective_compute(
                    "AllGather", AluOpType.bypass,
                    replica_groups=[list(range(ncores))],
                    ins=[cc_in[:]], outs=[cc_out[:]])
                nc.sync.dma_start(
                    land2[:].rearrange("p (r c) -> p r c", c=2),
                    cc_out[:].rearrange("(r p) c -> p r c", r=ncores))
            else:
              with tc.tile_critical():
                g = nc.gpsimd
                g.tensor_copy(mysums[:], sums_sb[:])
                g.bir_kernel_barrier_wait([list(range(ncores))])
                pid = g.partition_id()
                g.remote_dma_broadcast(
                    out_ap=land[:, bass.ds(pid * 2, 2)],
                    in_ap=mysums[:],
                    remote_sem=rsem, local_sem=lsem,
                    rdests=[(0, k) for k in range(ncores)],
                ).then_inc(psem, 1)
                g.wait_ge(psem, 1)
                g.trigger_dma(count=1)
                g.wait_ge(rsem, 16)
                g.wait_ge(lsem, 16)
                g.tensor_copy(land2[:], land[:])
            # (the 1/(ncores*free) mean scale is folded into sew1l/sew1g
            # on the host, so the raw sums feed the SE matmul directly)
            mean_sb = small.tile([C, 2, 1], f32)
            nc.vector.tensor_reduce(
                mean_sb, land2[:].rearrange("p (r c) -> p c r", c=2),
                AxisListType.X, AluOpType.add)

            # ---- mask pipeline (tiny) ----
            # FCNet: m1 = sigmoid(w2^T relu(w1^T mask + b1) + b2), column form
            ps1 = ps_small.tile([C, 1], f32, tag="ps1")
            nc.tensor.matmul(ps1, w1_sb, maskv_sb, start=True, stop=True)
            h1 = small.tile([C, 1], f32)
            nc.vector.tensor_scalar(h1, ps1, b1_sb, 0.0,
                                    AluOpType.add, AluOpType.max)
            ps2 = ps_small.tile([C, 1], f32, tag="ps1")
            nc.tensor.matmul(ps2, w2_sb, h1, start=True, stop=True)
            m1 = small.tile([C, 1], f32)
            nc.scalar.activation(m1, ps2, AF.Sigmoid, bias=b2_sb)

            # SE: m2 = sigmoid(se_w2^T relu(se_w1^T pooled + se_b1) + se_b2)
            ps3 = ps_small.tile([SE_HID, 1], f32, tag="ps1")
            nc.tensor.matmul(ps3, sew1l_sb, mean_sb[:, 0, :], start=True,
                             stop=False)
            nc.tensor.matmul(ps3, sew1g_sb, mean_sb[:, 1, :], start=False,
                             stop=True)
            hse = small.tile([SE_HID, 1], f32)
            nc.vector.tensor_scalar(hse, ps3, seb1_sb, 0.0,
                                    AluOpType.add, AluOpType.max)
            ps4 = ps_small.tile([C, 1], f32, tag="ps1")
            nc.tensor.matmul(ps4, sew2_sb, hse, start=True, stop=True)
            m2 = small.tile([C, 1], f32)
            nc.scalar.activation(m2, ps4, AF.Sigmoid, bias=seb2_sb)

            m_col = small.tile([C, 1], f32)
            nc.vector.tensor_tensor(m_col, m1, m2, AluOpType.mult)

            # transpose m -> row layout (also the out_m output)
            ps_mrow = ps_small.tile([1, C], f32, tag="ps1")
            nc.tensor.transpose(ps_mrow, m_col, ident)
            m_row = small.tile([1, C], f32)
            nc.vector.tensor_copy(m_row, ps_mrow)
            nc.sync.dma_start(out_m[:], m_row)

            # selection: cnt[c] = #{c' : m[c'] > m[c]};  s = (cnt < 64)
            ps_bc = ps_small.tile([C, C], f32, tag="ps_bc")
            nc.tensor.matmul(ps_bc, ones_row, m_row, start=True, stop=True)
            gt = small.tile([C, C], f32)
            nc.vector.tensor_scalar(gt, ps_bc, m_col, None, AluOpType.is_gt)
            cnt_col = small.tile([C, 1], f32)
            nc.vector.tensor_reduce(cnt_col, gt, AxisListType.X,
                                    AluOpType.add)
            s_col = small.tile([C, 1], f32)
            nc.vector.tensor_scalar(s_col, cnt_col, float(HALF), None,
                                    AluOpType.is_lt)
            oms = small.tile([C, 1], f32)   # 1 - s
            nc.vector.tensor_scalar(oms, s_col, -1.0, 1.0,
                                    AluOpType.mult, AluOpType.add)

            # ranks: exclusive prefix sum of s along channel index
            ps_srow = ps_small.tile([1, C], f32, tag="ps1")
            nc.tensor.transpose(ps_srow, s_col, ident)
            s_row = small.tile([1, C], f32)
            nc.vector.tensor_copy(s_row, ps_srow)
            incl = small.tile([1, C], f32)
            nc.vector.tensor_tensor_scan(incl, s_row, zeros_row, 0.0,
                                         AluOpType.add, AluOpType.add)
            rank_row = small.tile([1, C], f32)
            nc.vector.tensor_sub(rank_row, incl, s_row)
            ps_rcol = ps_small.tile([C, 1], f32, tag="ps1")
            nc.tensor.transpose(ps_rcol, rank_row, ident[:1, :1])
            rank_col = small.tile([C, 1], f32)
            nc.vector.tensor_copy(rank_col, ps_rcol)

            # selection matrix P[c, j] = s[c] * (rank[c] == j)  -> and P^T
            P = small.tile([C, HALF], f32)
            nc.vector.tensor_scalar(P, iota_j, rank_col, s_col,
                                    AluOpType.is_equal, AluOpType.mult)
            ps_pt = ps_small.tile([HALF, C], f32, tag="ps1")
            nc.tensor.transpose(ps_pt, P, ident)
            PT = small.tile([HALF, C], f32)
            nc.vector.tensor_copy(PT, ps_pt)

            # Z2 = conv2_w^T P^T ; Z1 = conv1_w^T P^T  (cw_sb = [c2w | c1w])
            ps_z2 = ps_small.tile([HALF, C], f32, tag="ps1")
            nc.tensor.matmul(ps_z2, cw_sb[:, 0:HALF], PT, start=True,
                             stop=True)
            z2_sb = small.tile([HALF, C], f32)
            nc.vector.tensor_copy(z2_sb, ps_z2)
            ps_z1 = ps_small.tile([HALF, C], f32, tag="ps1")
            nc.tensor.matmul(ps_z1, cw_sb[:, HALF:C], PT, start=True,
                             stop=True)
            z1_sb = small.tile([HALF, C], f32)
            nc.vector.tensor_copy(z1_sb, ps_z1)
            ps_a2 = ps_small.tile([C, C], f32, tag="ps_bc")
            nc.tensor.matmul(ps_a2, PT, z2_sb, start=True, stop=True)
            A2T = const.tile([C, C], f32r, tag="A2T")
            nc.vector.tensor_copy(A2T, ps_a2)
            ps_a1 = ps_small.tile([C, C], f32, tag="ps_bc")
            nc.tensor.matmul(ps_a1, PT, z1_sb, start=True, stop=True)
            A1T = const.tile([C, C], f32r, tag="A1T")
            nc.vector.tensor_copy(A1T, ps_a1)

            ps_beta = ps_small.tile([C, 2], f32, tag="ps1")
            nc.tensor.matmul(ps_beta, PT, cb_sb, start=True, stop=True)
            beta = small.tile([C, 2], f32)
            nc.vector.tensor_copy(beta, ps_beta)
            beta2 = beta[:, 0:1]
            beta1 = beta[:, 1:2]

            # ---- Phase B: streamed exchange (stashed tiles first) ----
            from concourse.tile import add_dep_helper

            order = list(range(nt - n_stash, nt)) + list(range(nt - n_stash))
            n_deferred = 0
            for t in order:
                if t in stash_tiles:
                    li, gi = stash_tiles[t]
                else:
                    li = io_in.tile([C, tile], f32r, tag="io_lst")
                    d1 = nc.sync.dma_start(li, lst[:, t * tile:(t + 1) * tile])
                    gi = io_in.tile([C, tile], f32r, tag="io_gui")
                    d2 = nc.sync.dma_start(gi, gui[:, t * tile:(t + 1) * tile])
                    if n_deferred < 4:
                        # keep phase-A loads ahead of the re-stream prefetch
                        # so the sums (and the exchange) trigger early
                        add_dep_helper(d1.ins, last_a_load.ins, sync=False,
                                       reason="defer phase-B prefetch")
                        add_dep_helper(d2.ins, last_a_load.ins, sync=False,
                                       reason="defer phase-B prefetch")
                        n_deferred += 1
                for j in range(mm_per_tile):
                    sl = slice(j * mm, (j + 1) * mm)
                    c0 = t * tile + j * mm
                    pl = ps_big.tile([C, mm], f32, tag="ps_l")
                    nc.tensor.matmul(pl, A2T[:], gi[:, sl],
                                     start=True, stop=True)
                    lo = b_out.tile([C, mm], f32, tag="o_lst")
                    nc.scalar.activation(lo, li[:, sl], AF.Identity,
                                         bias=beta2, scale=oms)
                    nc.vector.tensor_add(lo, lo, pl)
                    nc.sync.dma_start(out_lst[:, c0:c0 + mm], lo)

                    pg = ps_big.tile([C, mm], f32, tag="ps_g")
                    nc.tensor.matmul(pg, A1T[:], li[:, sl],
                                     start=True, stop=True)
                    go = b_out.tile([C, mm], f32, tag="o_gui")
                    nc.scalar.activation(go, gi[:, sl], AF.Identity,
                                         bias=beta1, scale=oms)
                    nc.vector.tensor_add(go, go, pg)
                    nc.sync.dma_start(out_gui[:, c0:c0 + mm], go)

    nc.compile()
    return nc


def make_in_maps(lst, gui, mask, w1, b1, w2, b2, se_w1, se_b1, se_w2, se_b2,
                 conv1_w, conv1_b, conv2_w, conv2_b, ncores=NCORES,
                 hshard=HSHARD, mean_scale=None):
    """Shard the full inputs into per-core input maps."""
    f = np.float32
    if mean_scale is None:
        mean_scale = 1.0 / (ncores * hshard * W)
    lst = np.asarray(lst, f)
    gui = np.asarray(gui, f)
    cw = np.concatenate([np.asarray(conv2_w, f), np.asarray(conv1_w, f)],
                        axis=1)
    cb = np.stack([np.asarray(conv2_b, f), np.asarray(conv1_b, f)], axis=1)
    shared = {
        "maskv": np.ascontiguousarray(np.asarray(mask, f).reshape(1, MD).T),
        "w1": np.ascontiguousarray(np.asarray(w1, f)),
        "b1": np.ascontiguousarray(np.asarray(b1, f).reshape(C, 1)),
        "w2": np.ascontiguousarray(np.asarray(w2, f)),
        "b2": np.ascontiguousarray(np.asarray(b2, f).reshape(C, 1)),
        "sew1l": np.ascontiguousarray(np.asarray(se_w1, f)[:C] * f(mean_scale)),
        "sew1g": np.ascontiguousarray(np.asarray(se_w1, f)[C:] * f(mean_scale)),
        "seb1": np.ascontiguousarray(np.asarray(se_b1, f).reshape(SE_HID, 1)),
        "sew2": np.ascontiguousarray(np.asarray(se_w2, f)),
        "seb2": np.ascontiguousarray(np.asarray(se_b2, f).reshape(C, 1)),
        "cw_cat": np.ascontiguousarray(cw),
        "cb_cat": np.ascontiguousarray(cb),
    }
    in_maps = []
    for i in range(ncores):
        sl = slice(i * hshard, (i + 1) * hshard)
        in_maps.append({
            "lst": np.ascontiguousarray(lst[0, :, sl, :]).reshape(C, -1),
            "gui": np.ascontiguousarray(gui[0, :, sl, :]).reshape(C, -1),
            **shared,
        })
    return in_maps


_NC_CACHE = {}


def kernel(lst, gui, mask, w1, b1, w2, b2, se_w1, se_b1, se_w2, se_b2,
           conv1_w, conv1_b, conv2_w, conv2_b):
    from concourse.bass_utils import run_bass_kernel_spmd

    if "nc" not in _NC_CACHE:
        _NC_CACHE["nc"] = build_nc()
    nc = _NC_CACHE["nc"]

    in_maps = make_in_maps(lst, gui, mask, w1, b1, w2, b2, se_w1, se_b1,
                           se_w2, se_b2, conv1_w, conv1_b, conv2_w, conv2_b)
    res = run_bass_kernel_spmd(nc, in_maps, core_ids=list(range(NCORES)))

    out_lst = np.empty((1, C, H, W), np.float32)
    out_gui = np.empty((1, C, H, W), np.float32)
    for i in range(NCORES):
        sl = slice(i * HSHARD, (i + 1) * HSHARD)
        out_lst[0, :, sl, :] = res.results[i]["out_lst"].reshape(C, HSHARD, W)
        out_gui[0, :, sl, :] = res.results[i]["out_gui"].reshape(C, HSHARD, W)
    m = res.results[0]["out_m"].reshape(1, C).copy()
    return out_lst, out_gui, m
